# revision 24
# baseline (speedup 1.0000x reference)
"""DepthRelationEmbedding Trainium2 kernel.

Math: out[h, n, hw] = relu( sum_d pos[n,hw,d] * W[d,h] + b[h] ) where pos is the
interleaved sin/cos embedding of delta[n,hw] = ln((relu(pd[n])+eps)/(dm[hw]+eps)).

Key identity: the embedding angle separates: angle_k(n,hw) = A_k(n) - C_k(hw)
with A_k = s_k*ln(relu(pd)+eps), C_k = s_k*ln(dm+eps). Using angle addition the
(N, HW, 256) intermediate never exists:
  out[n,hw,h] = sum_k U[k,(n,h)]*cosC[k,hw] + V[k,(n,h)]*sinC[k,hw]
  U = sinA*We + cosA*Wo,  V = sinA*Wo - cosA*We   (We = W[0::2], Wo = W[1::2])
which is one (256 x M) @ (256 x HW) matmul per core.

Trig arguments reach +-1600 rad; ACT Sin is only valid on [-pi, pi], so angles
are computed in "turns" (tau = angle/2pi) via a K=6 bf16-split outer-product
matmul (exact to ~2^-24), range-reduced with f = tau - rint(tau) (the DVE
fp32->int32 copy rounds to nearest on HW), and evaluated as
  sin(2pi tau) = Sin(2pi*f),  cos(2pi tau) = Sin(pi/2 - 2pi*|f|).

Sharding: SN x SH = 4 x 2 cores over (N, HW). Each core computes a full
[M=n_pad*8, hw_per] output block; host reassembles.
"""

import numpy as np

import sys

for p in ("/opt/trn_rl_repo", "/root/.axon_site/_ro/trn_rl_repo"):
    if p not in sys.path:
        sys.path.insert(0, p)

import ml_dtypes
from contextlib import ExitStack

from concourse import bacc, mybir, tile
from concourse.bass_utils import run_bass_kernel_spmd

F32 = mybir.dt.float32
F32R = mybir.dt.float32r
BF16 = mybir.dt.bfloat16
I32 = mybir.dt.int32
A = mybir.AluOpType
AF = mybir.ActivationFunctionType

# ---- problem constants (hardcoded; kernel.py must be self-contained) ----
N_TOT, H_DM, W_DM = 300, 24, 80
HW_TOT = H_DM * W_DM  # 1920
HEADS = 8
ED = 256  # embed dim
K = ED // 2  # 128 frequencies
EPS = 1e-5
SCALE = 100.0
TEMPERATURE = 10000.0
TWO_PI = 2.0 * np.pi

# ---- sharding config ----
SN, SH = 4, 2  # cores = SN * SH = 8
n_per = -(-N_TOT // SN)  # 75
n_pad = n_per + (n_per % 2)  # 76 (even, for clean [dm_rows, pd_cols] packing)
hw_per = HW_TOT // SH  # 960
dm_rows = 128 // SH  # 64
pd_cols = -(-n_pad // dm_rows)  # 2
M = n_pad * HEADS  # 608
CH = 480  # hw chunk width (psum-bank sized)
n_chunks = hw_per // CH
LPW = 15 + pd_cols  # logpack width

_m_tiles = []
_ms = 0
while _ms < M:
    _m_tiles.append((_ms, min(128, M - _ms)))
    _ms += 128


def _sigma_row():
    k = np.arange(K)
    dim_t = (TEMPERATURE ** (k.astype(np.float32) * 2.0 / ED)).astype(np.float32)
    sigma = (SCALE / dim_t.astype(np.float64)) / TWO_PI
    return sigma.astype(np.float32)[None, :]  # [1,128]


def _build_program():
    nc = bacc.Bacc("TRN2", target_bir_lowering=False, debug=False)

    pd_d = nc.dram_tensor("pdrow", [1, n_pad], F32, kind="ExternalInput")
    dm_d = nc.dram_tensor("dmtile", [dm_rows, 15], F32, kind="ExternalInput")
    wew_d = nc.dram_tensor("wew", [K, 2 * HEADS], F32, kind="ExternalInput")
    wow_d = nc.dram_tensor("wow", [K, 2 * HEADS], F32, kind="ExternalInput")
    bias_d = nc.dram_tensor("bias_rep", [128, 1], F32, kind="ExternalInput")
    out_d = nc.dram_tensor("out", [M, hw_per], F32, kind="ExternalOutput")
    sig_d = nc.inline_tensor(np.ascontiguousarray(_sigma_row()), name="sigma1")

    with tile.TileContext(nc) as tc, ExitStack() as ctx:
        sb = ctx.enter_context(tc.tile_pool(name="sb", bufs=1))
        sb2 = ctx.enter_context(tc.tile_pool(name="sb2", bufs=2))
        ps = ctx.enter_context(tc.tile_pool(name="ps", bufs=1, space="PSUM"))
        ps2 = ctx.enter_context(tc.tile_pool(name="ps2", bufs=2, space="PSUM"))
        pso = ctx.enter_context(tc.tile_pool(name="pso", bufs=5, space="PSUM"))

        # ---- constants ----
        def const_tile(val, tag):
            t = sb.tile((128, 1), F32, tag=tag)
            nc.vector.memset(t[:], val)
            return t

        twopi_c = const_tile(TWO_PI, "c_2pi")
        negtwopi_c = const_tile(-TWO_PI, "c_n2pi")
        halfpi_c = const_tile(np.pi / 2, "c_hpi")

        # trigger the natural_log ACT table load at kernel start so the real
        # Ln below doesn't stall on it
        lnwarm = sb.tile((128, 1), F32, tag="lnwarm")
        nc.scalar.activation(lnwarm[:], twopi_c[:], AF.Ln)

        # PE warmup: ~5us of dummy matmuls so HAM un-throttles (1.2->2.4 GHz)
        # before the real tau matmuls; they share the psa psum slot so the
        # tau_A matmul simply queues behind them.
        wa = sb.tile((128, 128), BF16, tag="wa")
        wb = sb.tile((128, 512), BF16, tag="wb")
        nc.gpsimd.memset(wa[:], 0)
        nc.gpsimd.memset(wb[:], 0)
        ps_warm = ps.tile((128, 512), F32, tag="psa")
        for _ in range(8):
            nc.tensor.matmul(ps_warm[:], wa[:], wb[:], start=True, stop=True)

        # input DMAs spread across engine queues; pd/dm first (critical path)
        pdr = sb.tile((1, n_pad), F32, tag="pdr")
        nc.sync.dma_start(pdr[:], pd_d[:])
        dmt = sb.tile((dm_rows, 15), F32, tag="dmt")
        nc.sync.dma_start(dmt[:], dm_d[:])
        lhs_s = sb.tile((1, K), F32, tag="lhs_s")
        nc.scalar.dma_start(lhs_s[:], sig_d[:])
        wew_t = sb.tile((K, 2 * HEADS), F32, tag="wew")
        nc.gpsimd.dma_start(wew_t[:], wew_d[:])
        wow_t = sb.tile((K, 2 * HEADS), F32, tag="wow")
        nc.scalar.dma_start(wow_t[:], wow_d[:])
        bias_t = sb.tile((128, 1), F32, tag="bias")
        nc.gpsimd.dma_start(bias_t[:], bias_d[:])

        # ---- logs. pd: relu,+eps,ln directly on its single-partition row
        #      (becomes rhs_a with no flatten); dm: +eps, ln, one flatten DMA ----
        _qeng = [nc.sync, nc.scalar, nc.gpsimd]
        nc.vector.tensor_scalar(pdr[:], pdr[:], 0.0, EPS, A.max, A.add)
        rhs_a = sb.tile((1, n_pad), F32, tag="rhs_a")
        nc.scalar.activation(rhs_a[:], pdr[:], AF.Ln)

        nc.vector.tensor_scalar(dmt[:], dmt[:], EPS, None, A.add)
        lnv = sb.tile((dm_rows, 15), F32, tag="lnv")
        nc.scalar.activation(lnv[:], dmt[:], AF.Ln)
        rhs_c = sb.tile((1, hw_per), F32, tag="rhs_c")
        nc.sync.dma_start(
            rhs_c[0:1, :].rearrange("r (p j) -> r p j", j=15), lnv[:]
        )

        # ---- helper: tau psum -> (sin, cos) via range reduction ----
        def reduce_and_trig(ps_t, width, sin_ap, cos_ap, tag, q_on_act=False):
            q = sb2.tile((K, width), I32, tag=f"q{tag}")
            if q_on_act:
                nc.scalar.activation(q[:], ps_t[:], AF.Copy)  # rint on HW
            else:
                nc.vector.tensor_copy(q[:], ps_t[:])  # rint on HW
            f = sb2.tile((K, width), F32, tag=f"f{tag}")
            nc.vector.tensor_tensor(f[:], ps_t[:], q[:], A.subtract)
            u = sb2.tile((K, width), F32, tag=f"u{tag}")
            ui = nc.vector.tensor_scalar(
                u[:].bitcast(I32), f[:].bitcast(I32), 0x7FFFFFFF, None, A.bitwise_and
            )
            si = nc.scalar.activation(sin_ap, f[:], AF.Sin, scale=twopi_c[:])
            nc.scalar.activation(
                cos_ap, u[:], AF.Sin, bias=halfpi_c[:], scale=negtwopi_c[:]
            )
            return ui, si

        # ---- A-side grid (sin and cos packed in one tile for 4-op T build) ----
        ps_a = ps.tile((K, n_pad), F32, tag="psa")
        nc.tensor.matmul(ps_a[:], lhs_s[:], rhs_a[:], start=True, stop=True)
        trigA = sb.tile((K, 2 * n_pad), F32, tag="trigA")
        ua_i, _ = reduce_and_trig(ps_a, n_pad, trigA[:, 0:n_pad], trigA[:, n_pad:], "a")

        # ---- T build: U = sinA*We + cosA*Wo, V = sinA*Wo - cosA*We ----
        # products as one [K, 2, n_pad, HEADS] broadcast multiply per (U, V)
        U = sb.tile((K, M), F32R, tag="U")
        V = sb.tile((K, M), F32R, tag="V")
        tmp1 = sb.tile((K, 2 * M), F32, tag="tmp1")
        tmp2 = sb.tile((K, 2 * M), F32, tag="tmp2")

        def trig_bc():  # [K, 2*n_pad] -> [K, 2, n_pad, HEADS]
            return (
                trigA[:]
                .rearrange("p (s n) -> p s n", s=2)
                .unsqueeze(3)
                .to_broadcast((K, 2, n_pad, HEADS))
            )

        def w_bc(t):  # [K, 2*HEADS] -> [K, 2, n_pad, HEADS]
            return (
                t[:]
                .rearrange("p (s h) -> p s h", s=2)
                .unsqueeze(2)
                .to_broadcast((K, 2, n_pad, HEADS))
            )

        def r4(t):  # [K, 2*M] viewed as [K, 2, n_pad, HEADS]
            return t[:].rearrange("p (s n h) -> p s n h", s=2, h=HEADS)

        HN = 48  # first 48 n-values cover m-tiles 0..2 (384 = 48*8)
        def _trig_bc(n0, n1):
            return (
                trigA[:]
                .rearrange("p (s n) -> p s n", s=2)[:, :, n0:n1]
                .unsqueeze(3)
                .to_broadcast((K, 2, n1 - n0, HEADS))
            )

        def _w_bc(t, n0, n1):
            return (
                t[:]
                .rearrange("p (s h) -> p s h", s=2)
                .unsqueeze(2)
                .to_broadcast((K, 2, n1 - n0, HEADS))
            )

        def _r4(t, n0, n1):
            return t[:].rearrange("p (s n h) -> p s n h", s=2, h=HEADS)[:, :, n0:n1, :]

        def build_half(out_t, tmp_t, w_t, op, n0, n1):
            p = nc.vector.tensor_tensor(
                _r4(tmp_t, n0, n1), _trig_bc(n0, n1), _w_bc(w_t, n0, n1), A.mult
            )
            c0, c1 = n0 * HEADS, n1 * HEADS
            o = nc.vector.tensor_tensor(
                out_t[:, c0:c1], tmp_t[:, c0 : c1], tmp_t[:, M + c0 : M + c1], op
            )
            return p, o

        def build_U(n0=0, n1=n_pad):
            return build_half(U, tmp1, wew_t, A.add, n0, n1)

        def build_V(n0=0, n1=n_pad):
            return build_half(V, tmp2, wow_t, A.subtract, n0, n1)

        # ---- C-side grids: tau matmuls early (PE idle), reduction chunk0
        #      before T-build so ACT trig overlaps DVE T-build ----
        cs_sin = sb.tile((K, hw_per), F32R, tag="cs_sin")
        cs_cos = sb.tile((K, hw_per), F32R, tag="cs_cos")
        ps_cs = []
        tau_insts = []
        for ci in range(n_chunks):
            sl = slice(ci * CH, (ci + 1) * CH)
            ps_c = ps2.tile((K, CH), F32, tag="psc")
            ti_ = nc.tensor.matmul(ps_c[:], lhs_s[:], rhs_c[:, sl], start=True, stop=True)
            tau_insts.append(ti_)
            ps_cs.append(ps_c)
        from concourse.tile_rust import add_dep_helper

        # HAM-warmth fillers pinned (same-engine order dep) after the last tau
        # matmul so they fill the PE gap before the main matmuls
        last_tau = tau_insts[-1]
        for _ in range(3):
            fi = nc.tensor.matmul(ps_warm[:], wa[:], wb[:], start=True, stop=True)
            add_dep_helper(fi.ins, last_tau.ins, sync=False, reason="warmth filler")
            last_tau = fi

        u0_i, _ = reduce_and_trig(
            ps_cs[0], CH, cs_sin[:, 0:CH], cs_cos[:, 0:CH], "c", q_on_act=True
        )
        p1a, _ = build_U(0, HN)
        add_dep_helper(p1a.ins, u0_i.ins, sync=False, reason="order: red0 < Ua")
        p1b, _ = build_U(HN, n_pad)
        u1_i = None
        for ci in range(1, n_chunks):
            sl = slice(ci * CH, (ci + 1) * CH)
            u1_i, _ = reduce_and_trig(
                ps_cs[ci], CH, cs_sin[:, sl], cs_cos[:, sl], "c", q_on_act=True
            )
        p2a, _ = build_V(0, HN)
        if u1_i is not None:
            add_dep_helper(p2a.ins, u1_i.ins, sync=False, reason="order: red1 < Va")
        build_V(HN, n_pad)


        # ---- main matmuls + relu + store: per chunk emit all U-matmuls
        #      first, then all V-matmuls (V is built later than U) ----
        obs = {}
        for ci in range(n_chunks):
            sl = slice(ci * CH, (ci + 1) * CH)
            ps_os = []
            for mi, (ms, mr) in enumerate(_m_tiles):
                ps_o = pso.tile((128, CH), F32, tag="pso")
                ps_os.append(ps_o)
                nc.tensor.matmul(
                    ps_o[:mr, :], U[:, ms : ms + mr], cs_cos[:, sl],
                    start=True, stop=False,
                )
            for mi, (ms, mr) in enumerate(_m_tiles):
                ps_o = ps_os[mi]
                nc.tensor.matmul(
                    ps_o[:mr, :], V[:, ms : ms + mr], cs_sin[:, sl],
                    start=False, stop=True,
                )
                if ci == 0:
                    ob_new = sb.tile((128, hw_per), F32, tag=f"ob{mi}")
                    obs[mi] = ob_new
                ob = obs[mi]
                ti = ci * len(_m_tiles) + mi
                if ti % 2 == 0:
                    nc.scalar.activation(
                        ob[:mr, sl], ps_o[:mr, :], AF.Relu, bias=bias_t[0:mr]
                    )
                else:
                    nc.vector.tensor_scalar(
                        ob[:mr, sl], ps_o[:mr, :], bias_t[0:mr], 0.0, A.add, A.max
                    )
                oq = nc.sync if ti % 2 == 0 else nc.scalar
                oq.dma_start(out_d[ms : ms + mr, sl], ob[:mr, sl])

    nc.finalize()
    return nc


_NC = None


def _get_nc():
    global _NC
    if _NC is None:
        _NC = _build_program()
    return _NC


def _make_in_maps(predict_depth, depth_map, W, b):
    pd = np.asarray(predict_depth, np.float32).reshape(N_TOT)
    dm = np.asarray(depth_map, np.float32).reshape(128, 15)
    W = np.asarray(W, np.float32)
    b = np.asarray(b, np.float32)
    we = W[0::2, :]
    wo = W[1::2, :]
    wew = np.ascontiguousarray(np.stack([we, wo], axis=1).reshape(K, 2 * HEADS))
    wow = np.ascontiguousarray(np.stack([wo, we], axis=1).reshape(K, 2 * HEADS))
    bias_rep = np.ascontiguousarray(np.tile(b, 16)[:, None])

    in_maps = []
    for c in range(SN * SH):
        ni, hi = c // SH, c % SH
        pd_sl = pd[ni * n_per : ni * n_per + n_per]
        pd_row = np.zeros((1, n_pad), np.float32)
        pd_row[0, : pd_sl.size] = pd_sl
        in_maps.append(
            {
                "pdrow": pd_row,
                "dmtile": np.ascontiguousarray(dm[hi * dm_rows : (hi + 1) * dm_rows]),
                "wew": wew,
                "wow": wow,
                "bias_rep": bias_rep,
            }
        )
    return in_maps


def _run(inputs, trace=False):
    nc = _get_nc()
    in_maps = _make_in_maps(**inputs)
    res = run_bass_kernel_spmd(
        nc, in_maps, core_ids=list(range(SN * SH)), trace=trace
    )
    out = np.empty((HEADS, N_TOT, HW_TOT), np.float32)
    for c in range(SN * SH):
        ni, hi = c // SH, c % SH
        blk = res.results[c]["out"].reshape(n_pad, HEADS, hw_per).transpose(1, 0, 2)
        n0 = ni * n_per
        n_cnt = min(n_per, N_TOT - n0)
        out[:, n0 : n0 + n_cnt, hi * hw_per : (hi + 1) * hw_per] = blk[:, :n_cnt, :]
    return out, res


def kernel(predict_depth, depth_map, W, b):
    out, _ = _run(
        {"predict_depth": predict_depth, "depth_map": depth_map, "W": W, "b": b}
    )
    return out


# revision 25
# speedup vs baseline: 1.0837x; 1.0837x over previous
"""DepthRelationEmbedding Trainium2 kernel.

Math: out[h, n, hw] = relu( sum_d pos[n,hw,d] * W[d,h] + b[h] ) where pos is the
interleaved sin/cos embedding of delta[n,hw] = ln((relu(pd[n])+eps)/(dm[hw]+eps)).

Key identity: the embedding angle separates: angle_k(n,hw) = A_k(n) - C_k(hw)
with A_k = s_k*ln(relu(pd)+eps), C_k = s_k*ln(dm+eps). Using angle addition the
(N, HW, 256) intermediate never exists:
  out[n,hw,h] = sum_k U[k,(n,h)]*cosC[k,hw] + V[k,(n,h)]*sinC[k,hw]
  U = sinA*We + cosA*Wo,  V = sinA*Wo - cosA*We   (We = W[0::2], Wo = W[1::2])
which is one (256 x M) @ (256 x HW) matmul per core.

Trig arguments reach +-1600 rad; ACT Sin is only valid on [-pi, pi], so angles
are computed in "turns" (tau = angle/2pi) via a K=6 bf16-split outer-product
matmul (exact to ~2^-24), range-reduced with f = tau - rint(tau) (the DVE
fp32->int32 copy rounds to nearest on HW), and evaluated as
  sin(2pi tau) = Sin(2pi*f),  cos(2pi tau) = Sin(pi/2 - 2pi*|f|).

Sharding: SN x SH = 4 x 2 cores over (N, HW). Each core computes a full
[M=n_pad*8, hw_per] output block; host reassembles.
"""

import numpy as np

import sys

for p in ("/opt/trn_rl_repo", "/root/.axon_site/_ro/trn_rl_repo"):
    if p not in sys.path:
        sys.path.insert(0, p)

import ml_dtypes
from contextlib import ExitStack

from concourse import bacc, mybir, tile
from concourse.bass_utils import run_bass_kernel_spmd

F32 = mybir.dt.float32
F32R = mybir.dt.float32r
BF16 = mybir.dt.bfloat16
I32 = mybir.dt.int32
A = mybir.AluOpType
AF = mybir.ActivationFunctionType

# ---- problem constants (hardcoded; kernel.py must be self-contained) ----
N_TOT, H_DM, W_DM = 300, 24, 80
HW_TOT = H_DM * W_DM  # 1920
HEADS = 8
ED = 256  # embed dim
K = ED // 2  # 128 frequencies
EPS = 1e-5
SCALE = 100.0
TEMPERATURE = 10000.0
TWO_PI = 2.0 * np.pi

# ---- sharding config ----
SN, SH = 4, 2  # cores = SN * SH = 8
n_per = -(-N_TOT // SN)  # 75
n_pad = n_per + (n_per % 2)  # 76 (even, for clean [dm_rows, pd_cols] packing)
hw_per = HW_TOT // SH  # 960
dm_rows = 128 // SH  # 64
pd_cols = -(-n_pad // dm_rows)  # 2
M = n_pad * HEADS  # 608
CH = 480  # hw chunk width (psum-bank sized)
n_chunks = hw_per // CH
LPW = 15 + pd_cols  # logpack width

_m_tiles = []
_ms = 0
while _ms < M:
    _m_tiles.append((_ms, min(128, M - _ms)))
    _ms += 128


def _sigma_row():
    k = np.arange(K)
    dim_t = (TEMPERATURE ** (k.astype(np.float32) * 2.0 / ED)).astype(np.float32)
    sigma = (SCALE / dim_t.astype(np.float64)) / TWO_PI
    return sigma.astype(np.float32)[None, :]  # [1,128]


def _build_program():
    nc = bacc.Bacc("TRN2", target_bir_lowering=False, debug=False)

    pd_d = nc.dram_tensor("pdrow", [1, n_pad], F32, kind="ExternalInput")
    dm_d = nc.dram_tensor("dmtile", [dm_rows, 15], F32, kind="ExternalInput")
    wew_d = nc.dram_tensor("wew", [K, 2 * HEADS], F32, kind="ExternalInput")
    wow_d = nc.dram_tensor("wow", [K, 2 * HEADS], F32, kind="ExternalInput")
    bias_d = nc.dram_tensor("bias_rep", [128, 1], F32, kind="ExternalInput")
    out_d = nc.dram_tensor("out", [M, hw_per], F32, kind="ExternalOutput")
    sig_d = nc.inline_tensor(np.ascontiguousarray(_sigma_row()), name="sigma1")

    with tile.TileContext(nc) as tc, ExitStack() as ctx:
        sb = ctx.enter_context(tc.tile_pool(name="sb", bufs=1))
        sb2 = ctx.enter_context(tc.tile_pool(name="sb2", bufs=2))
        ps = ctx.enter_context(tc.tile_pool(name="ps", bufs=1, space="PSUM"))
        ps2 = ctx.enter_context(tc.tile_pool(name="ps2", bufs=2, space="PSUM"))
        pso = ctx.enter_context(tc.tile_pool(name="pso", bufs=5, space="PSUM"))

        # ---- constants ----
        def const_tile(val, tag):
            t = sb.tile((128, 1), F32, tag=tag)
            nc.vector.memset(t[:], val)
            return t

        twopi_c = const_tile(TWO_PI, "c_2pi")
        negtwopi_c = const_tile(-TWO_PI, "c_n2pi")
        halfpi_c = const_tile(np.pi / 2, "c_hpi")

        # trigger the natural_log ACT table load at kernel start so the real
        # Ln below doesn't stall on it
        lnwarm = sb.tile((128, 1), F32, tag="lnwarm")
        nc.scalar.activation(lnwarm[:], twopi_c[:], AF.Ln)

        # PE warmup: ~5us of dummy matmuls so HAM un-throttles (1.2->2.4 GHz)
        # before the real tau matmuls; they share the psa psum slot so the
        # tau_A matmul simply queues behind them.
        wa = sb.tile((128, 128), BF16, tag="wa")
        wb = sb.tile((128, 512), BF16, tag="wb")
        nc.gpsimd.memset(wa[:], 0)
        nc.gpsimd.memset(wb[:], 0)
        ps_warm = ps.tile((128, 512), F32, tag="psa")
        for _ in range(8):
            nc.tensor.matmul(ps_warm[:], wa[:], wb[:], start=True, stop=True)

        # input DMAs spread across engine queues; pd/dm first (critical path)
        pdr = sb.tile((1, n_pad), F32, tag="pdr")
        nc.sync.dma_start(pdr[:], pd_d[:])
        dmt = sb.tile((dm_rows, 15), F32, tag="dmt")
        nc.sync.dma_start(dmt[:], dm_d[:])
        lhs_s = sb.tile((1, K), F32, tag="lhs_s")
        nc.scalar.dma_start(lhs_s[:], sig_d[:])
        wew_t = sb.tile((K, 2 * HEADS), F32, tag="wew")
        nc.gpsimd.dma_start(wew_t[:], wew_d[:])
        wow_t = sb.tile((K, 2 * HEADS), F32, tag="wow")
        nc.scalar.dma_start(wow_t[:], wow_d[:])
        bias_t = sb.tile((128, 1), F32, tag="bias")
        nc.gpsimd.dma_start(bias_t[:], bias_d[:])

        # ---- logs. pd: relu,+eps,ln directly on its single-partition row
        #      (becomes rhs_a with no flatten); dm: +eps, ln, one flatten DMA ----
        _qeng = [nc.sync, nc.scalar, nc.gpsimd]
        nc.vector.tensor_scalar(pdr[:], pdr[:], 0.0, EPS, A.max, A.add)
        rhs_a = sb.tile((1, n_pad), F32, tag="rhs_a")
        nc.scalar.activation(rhs_a[:], pdr[:], AF.Ln)

        nc.vector.tensor_scalar(dmt[:], dmt[:], EPS, None, A.add)
        lnv = sb.tile((dm_rows, 15), F32, tag="lnv")
        nc.scalar.activation(lnv[:], dmt[:], AF.Ln)
        rhs_c = sb.tile((1, hw_per), F32, tag="rhs_c")
        nc.sync.dma_start(
            rhs_c[0:1, :].rearrange("r (p j) -> r p j", j=15), lnv[:]
        )

        # ---- helper: tau psum -> (sin, cos) via range reduction ----
        def reduce_and_trig(ps_t, width, sin_ap, cos_ap, tag, q_on_act=False):
            q = sb2.tile((K, width), I32, tag=f"q{tag}")
            if q_on_act:
                nc.scalar.activation(q[:], ps_t[:], AF.Copy)  # rint on HW
            else:
                nc.vector.tensor_copy(q[:], ps_t[:])  # rint on HW
            f = sb2.tile((K, width), F32, tag=f"f{tag}")
            nc.vector.tensor_tensor(f[:], ps_t[:], q[:], A.subtract)
            u = sb2.tile((K, width), F32, tag=f"u{tag}")
            ui = nc.vector.tensor_scalar(
                u[:].bitcast(I32), f[:].bitcast(I32), 0x7FFFFFFF, None, A.bitwise_and
            )
            si = nc.scalar.activation(sin_ap, f[:], AF.Sin, scale=twopi_c[:])
            nc.scalar.activation(
                cos_ap, u[:], AF.Sin, bias=halfpi_c[:], scale=negtwopi_c[:]
            )
            return ui, si

        # ---- A-side grid (sin and cos packed in one tile for 4-op T build) ----
        ps_a = ps.tile((K, n_pad), F32, tag="psa")
        nc.tensor.matmul(ps_a[:], lhs_s[:], rhs_a[:], start=True, stop=True)
        trigA = sb.tile((K, 2 * n_pad), F32, tag="trigA")
        ua_i, _ = reduce_and_trig(ps_a, n_pad, trigA[:, 0:n_pad], trigA[:, n_pad:], "a")

        # ---- T build: U = sinA*We + cosA*Wo, V = sinA*Wo - cosA*We ----
        # products as one [K, 2, n_pad, HEADS] broadcast multiply per (U, V)
        U = sb.tile((K, M), F32R, tag="U")
        V = sb.tile((K, M), F32R, tag="V")
        tmp1 = sb.tile((K, 2 * M), F32, tag="tmp1")
        tmp2 = sb.tile((K, 2 * M), F32, tag="tmp2")

        def trig_bc():  # [K, 2*n_pad] -> [K, 2, n_pad, HEADS]
            return (
                trigA[:]
                .rearrange("p (s n) -> p s n", s=2)
                .unsqueeze(3)
                .to_broadcast((K, 2, n_pad, HEADS))
            )

        def w_bc(t):  # [K, 2*HEADS] -> [K, 2, n_pad, HEADS]
            return (
                t[:]
                .rearrange("p (s h) -> p s h", s=2)
                .unsqueeze(2)
                .to_broadcast((K, 2, n_pad, HEADS))
            )

        def r4(t):  # [K, 2*M] viewed as [K, 2, n_pad, HEADS]
            return t[:].rearrange("p (s n h) -> p s n h", s=2, h=HEADS)

        HN = 48  # first 48 n-values cover m-tiles 0..2 (384 = 48*8)
        def _trig_bc(n0, n1):
            return (
                trigA[:]
                .rearrange("p (s n) -> p s n", s=2)[:, :, n0:n1]
                .unsqueeze(3)
                .to_broadcast((K, 2, n1 - n0, HEADS))
            )

        def _w_bc(t, n0, n1):
            return (
                t[:]
                .rearrange("p (s h) -> p s h", s=2)
                .unsqueeze(2)
                .to_broadcast((K, 2, n1 - n0, HEADS))
            )

        def _r4(t, n0, n1):
            return t[:].rearrange("p (s n h) -> p s n h", s=2, h=HEADS)[:, :, n0:n1, :]

        def build_half(out_t, tmp_t, w_t, op, n0, n1):
            p = nc.vector.tensor_tensor(
                _r4(tmp_t, n0, n1), _trig_bc(n0, n1), _w_bc(w_t, n0, n1), A.mult
            )
            c0, c1 = n0 * HEADS, n1 * HEADS
            o = nc.vector.tensor_tensor(
                out_t[:, c0:c1], tmp_t[:, c0 : c1], tmp_t[:, M + c0 : M + c1], op
            )
            return p, o

        def build_U(n0=0, n1=n_pad):
            return build_half(U, tmp1, wew_t, A.add, n0, n1)

        def build_V(n0=0, n1=n_pad):
            return build_half(V, tmp2, wow_t, A.subtract, n0, n1)

        # ---- C-side grids: tau matmuls early (PE idle), reduction chunk0
        #      before T-build so ACT trig overlaps DVE T-build ----
        cs_sin = sb.tile((K, hw_per), F32R, tag="cs_sin")
        cs_cos = sb.tile((K, hw_per), F32R, tag="cs_cos")
        ps_cs = []
        tau_insts = []
        for ci in range(n_chunks):
            sl = slice(ci * CH, (ci + 1) * CH)
            ps_c = ps2.tile((K, CH), F32, tag="psc")
            ti_ = nc.tensor.matmul(ps_c[:], lhs_s[:], rhs_c[:, sl], start=True, stop=True)
            tau_insts.append(ti_)
            ps_cs.append(ps_c)
        from concourse.tile_rust import add_dep_helper

        # HAM-warmth fillers pinned (same-engine order dep) after the last tau
        # matmul so they fill the PE gap before the main matmuls
        last_tau = tau_insts[-1]
        for _ in range(3):
            fi = nc.tensor.matmul(ps_warm[:], wa[:], wb[:], start=True, stop=True)
            add_dep_helper(fi.ins, last_tau.ins, sync=False, reason="warmth filler")
            last_tau = fi

        u0_i, _ = reduce_and_trig(
            ps_cs[0], CH, cs_sin[:, 0:CH], cs_cos[:, 0:CH], "c", q_on_act=True
        )
        p1a, _ = build_U()
        add_dep_helper(p1a.ins, u0_i.ins, sync=False, reason="order: red0 < U")
        u1_i = None
        for ci in range(1, n_chunks):
            sl = slice(ci * CH, (ci + 1) * CH)
            u1_i, _ = reduce_and_trig(
                ps_cs[ci], CH, cs_sin[:, sl], cs_cos[:, sl], "c", q_on_act=True
            )
        p2a, _ = build_V()
        if u1_i is not None:
            add_dep_helper(p2a.ins, u1_i.ins, sync=False, reason="order: red1 < V")


        # ---- main matmuls + relu + store: per chunk emit all U-matmuls
        #      first, then all V-matmuls (V is built later than U) ----
        obs = {}
        for ci in range(n_chunks):
            sl = slice(ci * CH, (ci + 1) * CH)
            ps_os = []
            for mi, (ms, mr) in enumerate(_m_tiles):
                ps_o = pso.tile((128, CH), F32, tag="pso")
                ps_os.append(ps_o)
                nc.tensor.matmul(
                    ps_o[:mr, :], U[:, ms : ms + mr], cs_cos[:, sl],
                    start=True, stop=False,
                )
            for mi, (ms, mr) in enumerate(_m_tiles):
                ps_o = ps_os[mi]
                nc.tensor.matmul(
                    ps_o[:mr, :], V[:, ms : ms + mr], cs_sin[:, sl],
                    start=False, stop=True,
                )
                if ci == 0:
                    ob_new = sb.tile((128, hw_per), F32, tag=f"ob{mi}")
                    obs[mi] = ob_new
                ob = obs[mi]
                ti = ci * len(_m_tiles) + mi
                if ti % 2 == 0:
                    nc.scalar.activation(
                        ob[:mr, sl], ps_o[:mr, :], AF.Relu, bias=bias_t[0:mr]
                    )
                else:
                    nc.vector.tensor_scalar(
                        ob[:mr, sl], ps_o[:mr, :], bias_t[0:mr], 0.0, A.add, A.max
                    )
                oq = nc.sync if ti % 2 == 0 else nc.scalar
                oq.dma_start(out_d[ms : ms + mr, sl], ob[:mr, sl])

    nc.finalize()
    return nc


_NC = None


def _get_nc():
    global _NC
    if _NC is None:
        _NC = _build_program()
    return _NC


def _make_in_maps(predict_depth, depth_map, W, b):
    pd = np.asarray(predict_depth, np.float32).reshape(N_TOT)
    dm = np.asarray(depth_map, np.float32).reshape(128, 15)
    W = np.asarray(W, np.float32)
    b = np.asarray(b, np.float32)
    we = W[0::2, :]
    wo = W[1::2, :]
    wew = np.ascontiguousarray(np.stack([we, wo], axis=1).reshape(K, 2 * HEADS))
    wow = np.ascontiguousarray(np.stack([wo, we], axis=1).reshape(K, 2 * HEADS))
    bias_rep = np.ascontiguousarray(np.tile(b, 16)[:, None])

    in_maps = []
    for c in range(SN * SH):
        ni, hi = c // SH, c % SH
        pd_sl = pd[ni * n_per : ni * n_per + n_per]
        pd_row = np.zeros((1, n_pad), np.float32)
        pd_row[0, : pd_sl.size] = pd_sl
        in_maps.append(
            {
                "pdrow": pd_row,
                "dmtile": np.ascontiguousarray(dm[hi * dm_rows : (hi + 1) * dm_rows]),
                "wew": wew,
                "wow": wow,
                "bias_rep": bias_rep,
            }
        )
    return in_maps


def _run(inputs, trace=False):
    nc = _get_nc()
    in_maps = _make_in_maps(**inputs)
    res = run_bass_kernel_spmd(
        nc, in_maps, core_ids=list(range(SN * SH)), trace=trace
    )
    out = np.empty((HEADS, N_TOT, HW_TOT), np.float32)
    for c in range(SN * SH):
        ni, hi = c // SH, c % SH
        blk = res.results[c]["out"].reshape(n_pad, HEADS, hw_per).transpose(1, 0, 2)
        n0 = ni * n_per
        n_cnt = min(n_per, N_TOT - n0)
        out[:, n0 : n0 + n_cnt, hi * hw_per : (hi + 1) * hw_per] = blk[:, :n_cnt, :]
    return out, res


def kernel(predict_depth, depth_map, W, b):
    out, _ = _run(
        {"predict_depth": predict_depth, "depth_map": depth_map, "W": W, "b": b}
    )
    return out


# revision 26
# speedup vs baseline: 1.1828x; 1.0914x over previous
"""DepthRelationEmbedding Trainium2 kernel.

Math: out[h, n, hw] = relu( sum_d pos[n,hw,d] * W[d,h] + b[h] ) where pos is the
interleaved sin/cos embedding of delta[n,hw] = ln((relu(pd[n])+eps)/(dm[hw]+eps)).

Key identity: the embedding angle separates: angle_k(n,hw) = A_k(n) - C_k(hw)
with A_k = s_k*ln(relu(pd)+eps), C_k = s_k*ln(dm+eps). Using angle addition the
(N, HW, 256) intermediate never exists:
  out[n,hw,h] = sum_k U[k,(n,h)]*cosC[k,hw] + V[k,(n,h)]*sinC[k,hw]
  U = sinA*We + cosA*Wo,  V = sinA*Wo - cosA*We   (We = W[0::2], Wo = W[1::2])
which is one (256 x M) @ (256 x HW) matmul per core.

Trig arguments reach +-1600 rad; ACT Sin is only valid on [-pi, pi], so angles
are computed in "turns" (tau = angle/2pi) via a K=6 bf16-split outer-product
matmul (exact to ~2^-24), range-reduced with f = tau - rint(tau) (the DVE
fp32->int32 copy rounds to nearest on HW), and evaluated as
  sin(2pi tau) = Sin(2pi*f),  cos(2pi tau) = Sin(pi/2 - 2pi*|f|).

Sharding: SN x SH = 4 x 2 cores over (N, HW). Each core computes a full
[M=n_pad*8, hw_per] output block; host reassembles.
"""

import numpy as np

import sys

for p in ("/opt/trn_rl_repo", "/root/.axon_site/_ro/trn_rl_repo"):
    if p not in sys.path:
        sys.path.insert(0, p)

import ml_dtypes
from contextlib import ExitStack

from concourse import bacc, mybir, tile
from concourse.bass_utils import run_bass_kernel_spmd

F32 = mybir.dt.float32
F32R = mybir.dt.float32r
BF16 = mybir.dt.bfloat16
I32 = mybir.dt.int32
A = mybir.AluOpType
AF = mybir.ActivationFunctionType

# ---- problem constants (hardcoded; kernel.py must be self-contained) ----
N_TOT, H_DM, W_DM = 300, 24, 80
HW_TOT = H_DM * W_DM  # 1920
HEADS = 8
ED = 256  # embed dim
K = ED // 2  # 128 frequencies
EPS = 1e-5
SCALE = 100.0
TEMPERATURE = 10000.0
TWO_PI = 2.0 * np.pi

# ---- sharding config ----
SN, SH = 4, 2  # cores = SN * SH = 8
n_per = -(-N_TOT // SN)  # 75
n_pad = n_per + (n_per % 2)  # 76 (even, for clean [dm_rows, pd_cols] packing)
hw_per = HW_TOT // SH  # 960
dm_rows = 128 // SH  # 64
pd_cols = -(-n_pad // dm_rows)  # 2
M = n_pad * HEADS  # 608
CH = 480  # hw chunk width (psum-bank sized)
n_chunks = hw_per // CH
LPW = 15 + pd_cols  # logpack width

_m_tiles = []
_ms = 0
while _ms < M:
    _m_tiles.append((_ms, min(128, M - _ms)))
    _ms += 128


def _sigma_row():
    k = np.arange(K)
    dim_t = (TEMPERATURE ** (k.astype(np.float32) * 2.0 / ED)).astype(np.float32)
    sigma = (SCALE / dim_t.astype(np.float64)) / TWO_PI
    return sigma.astype(np.float32)[None, :]  # [1,128]


def _build_program():
    nc = bacc.Bacc("TRN2", target_bir_lowering=False, debug=False)

    pd_d = nc.dram_tensor("pdrow", [1, n_pad], F32, kind="ExternalInput")
    dm_d = nc.dram_tensor("dmtile", [dm_rows, 15], F32, kind="ExternalInput")
    wew_d = nc.dram_tensor("wew", [K, 2 * HEADS], F32, kind="ExternalInput")
    wow_d = nc.dram_tensor("wow", [K, 2 * HEADS], F32, kind="ExternalInput")
    bias_d = nc.dram_tensor("bias_rep", [128, 1], F32, kind="ExternalInput")
    out_d = nc.dram_tensor("out", [M, hw_per], F32, kind="ExternalOutput")
    sig_d = nc.inline_tensor(np.ascontiguousarray(_sigma_row()), name="sigma1")

    with tile.TileContext(nc) as tc, ExitStack() as ctx:
        sb = ctx.enter_context(tc.tile_pool(name="sb", bufs=1))
        sb2 = ctx.enter_context(tc.tile_pool(name="sb2", bufs=2))
        ps = ctx.enter_context(tc.tile_pool(name="ps", bufs=1, space="PSUM"))
        ps2 = ctx.enter_context(tc.tile_pool(name="ps2", bufs=2, space="PSUM"))
        pso = ctx.enter_context(tc.tile_pool(name="pso", bufs=5, space="PSUM"))

        # ---- constants ----
        def const_tile(val, tag):
            t = sb.tile((128, 1), F32, tag=tag)
            nc.vector.memset(t[:], val)
            return t

        twopi_c = const_tile(TWO_PI, "c_2pi")
        negtwopi_c = const_tile(-TWO_PI, "c_n2pi")
        halfpi_c = const_tile(np.pi / 2, "c_hpi")

        # trigger the natural_log ACT table load at kernel start so the real
        # Ln below doesn't stall on it
        lnwarm = sb.tile((128, 1), F32, tag="lnwarm")
        nc.scalar.activation(lnwarm[:], twopi_c[:], AF.Ln)

        # PE warmup: ~5us of dummy matmuls so HAM un-throttles (1.2->2.4 GHz)
        # before the real tau matmuls; they share the psa psum slot so the
        # tau_A matmul simply queues behind them.
        wa = sb.tile((128, 128), BF16, tag="wa")
        wb = sb.tile((128, 512), BF16, tag="wb")
        nc.gpsimd.memset(wa[:], 0)
        nc.gpsimd.memset(wb[:], 0)
        ps_warm = ps.tile((128, 512), F32, tag="psa")
        for _ in range(9):
            nc.tensor.matmul(ps_warm[:], wa[:], wb[:], start=True, stop=True)

        # input DMAs spread across engine queues; pd/dm first (critical path)
        pdr = sb.tile((1, n_pad), F32, tag="pdr")
        nc.sync.dma_start(pdr[:], pd_d[:])
        dmt = sb.tile((dm_rows, 15), F32, tag="dmt")
        nc.sync.dma_start(dmt[:], dm_d[:])
        lhs_s = sb.tile((1, K), F32, tag="lhs_s")
        nc.scalar.dma_start(lhs_s[:], sig_d[:])
        wew_t = sb.tile((K, 2 * HEADS), F32, tag="wew")
        nc.gpsimd.dma_start(wew_t[:], wew_d[:])
        wow_t = sb.tile((K, 2 * HEADS), F32, tag="wow")
        nc.scalar.dma_start(wow_t[:], wow_d[:])
        bias_t = sb.tile((128, 1), F32, tag="bias")
        nc.gpsimd.dma_start(bias_t[:], bias_d[:])

        # ---- logs. pd: relu,+eps,ln directly on its single-partition row
        #      (becomes rhs_a with no flatten); dm: +eps, ln, one flatten DMA ----
        _qeng = [nc.sync, nc.scalar, nc.gpsimd]
        nc.vector.tensor_scalar(pdr[:], pdr[:], 0.0, EPS, A.max, A.add)
        rhs_a = sb.tile((1, n_pad), F32, tag="rhs_a")
        nc.scalar.activation(rhs_a[:], pdr[:], AF.Ln)

        nc.vector.tensor_scalar(dmt[:], dmt[:], EPS, None, A.add)
        lnv = sb.tile((dm_rows, 15), F32, tag="lnv")
        nc.scalar.activation(lnv[:], dmt[:], AF.Ln)
        rhs_c = sb.tile((1, hw_per), F32, tag="rhs_c")
        nc.sync.dma_start(
            rhs_c[0:1, :].rearrange("r (p j) -> r p j", j=15), lnv[:]
        )

        # ---- helper: tau psum -> (sin, cos) via range reduction ----
        def reduce_and_trig(ps_t, width, sin_ap, cos_ap, tag, q_on_act=False):
            q = sb2.tile((K, width), I32, tag=f"q{tag}")
            if q_on_act:
                nc.scalar.activation(q[:], ps_t[:], AF.Copy)  # rint on HW
            else:
                nc.vector.tensor_copy(q[:], ps_t[:])  # rint on HW
            f = sb2.tile((K, width), F32, tag=f"f{tag}")
            nc.vector.tensor_tensor(f[:], ps_t[:], q[:], A.subtract)
            u = sb2.tile((K, width), F32, tag=f"u{tag}")
            ui = nc.vector.tensor_scalar(
                u[:].bitcast(I32), f[:].bitcast(I32), 0x7FFFFFFF, None, A.bitwise_and
            )
            si = nc.scalar.activation(sin_ap, f[:], AF.Sin, scale=twopi_c[:])
            nc.scalar.activation(
                cos_ap, u[:], AF.Sin, bias=halfpi_c[:], scale=negtwopi_c[:]
            )
            return ui, si

        # ---- A-side grid (sin and cos packed in one tile for 4-op T build) ----
        ps_a = ps.tile((K, n_pad), F32, tag="psa")
        nc.tensor.matmul(ps_a[:], lhs_s[:], rhs_a[:], start=True, stop=True)
        trigA = sb.tile((K, 2 * n_pad), F32, tag="trigA")
        ua_i, _ = reduce_and_trig(ps_a, n_pad, trigA[:, 0:n_pad], trigA[:, n_pad:], "a")

        # ---- T build: U = sinA*We + cosA*Wo, V = sinA*Wo - cosA*We ----
        # products as one [K, 2, n_pad, HEADS] broadcast multiply per (U, V)
        U = sb.tile((K, M), F32R, tag="U")
        V = sb.tile((K, M), F32R, tag="V")
        tmp1 = sb.tile((K, 2 * M), F32, tag="tmp1")
        tmp2 = sb.tile((K, 2 * M), F32, tag="tmp2")

        def trig_bc():  # [K, 2*n_pad] -> [K, 2, n_pad, HEADS]
            return (
                trigA[:]
                .rearrange("p (s n) -> p s n", s=2)
                .unsqueeze(3)
                .to_broadcast((K, 2, n_pad, HEADS))
            )

        def w_bc(t):  # [K, 2*HEADS] -> [K, 2, n_pad, HEADS]
            return (
                t[:]
                .rearrange("p (s h) -> p s h", s=2)
                .unsqueeze(2)
                .to_broadcast((K, 2, n_pad, HEADS))
            )

        def r4(t):  # [K, 2*M] viewed as [K, 2, n_pad, HEADS]
            return t[:].rearrange("p (s n h) -> p s n h", s=2, h=HEADS)

        HN = 48  # first 48 n-values cover m-tiles 0..2 (384 = 48*8)
        def _trig_bc(n0, n1):
            return (
                trigA[:]
                .rearrange("p (s n) -> p s n", s=2)[:, :, n0:n1]
                .unsqueeze(3)
                .to_broadcast((K, 2, n1 - n0, HEADS))
            )

        def _w_bc(t, n0, n1):
            return (
                t[:]
                .rearrange("p (s h) -> p s h", s=2)
                .unsqueeze(2)
                .to_broadcast((K, 2, n1 - n0, HEADS))
            )

        def _r4(t, n0, n1):
            return t[:].rearrange("p (s n h) -> p s n h", s=2, h=HEADS)[:, :, n0:n1, :]

        def build_half(out_t, tmp_t, w_t, op, n0, n1):
            p = nc.vector.tensor_tensor(
                _r4(tmp_t, n0, n1), _trig_bc(n0, n1), _w_bc(w_t, n0, n1), A.mult
            )
            c0, c1 = n0 * HEADS, n1 * HEADS
            o = nc.vector.tensor_tensor(
                out_t[:, c0:c1], tmp_t[:, c0 : c1], tmp_t[:, M + c0 : M + c1], op
            )
            return p, o

        def build_U(n0=0, n1=n_pad):
            return build_half(U, tmp1, wew_t, A.add, n0, n1)

        def build_V(n0=0, n1=n_pad):
            return build_half(V, tmp2, wow_t, A.subtract, n0, n1)

        # ---- C-side grids: tau matmuls early (PE idle), reduction chunk0
        #      before T-build so ACT trig overlaps DVE T-build ----
        cs_sin = sb.tile((K, hw_per), F32R, tag="cs_sin")
        cs_cos = sb.tile((K, hw_per), F32R, tag="cs_cos")
        ps_cs = []
        tau_insts = []
        for ci in range(n_chunks):
            sl = slice(ci * CH, (ci + 1) * CH)
            ps_c = ps2.tile((K, CH), F32, tag="psc")
            ti_ = nc.tensor.matmul(ps_c[:], lhs_s[:], rhs_c[:, sl], start=True, stop=True)
            tau_insts.append(ti_)
            ps_cs.append(ps_c)
        from concourse.tile_rust import add_dep_helper

        # HAM-warmth fillers (scheduler may hoist; still extends the busy window)
        for _ in range(3):
            nc.tensor.matmul(ps_warm[:], wa[:], wb[:], start=True, stop=True)

        u0_i, _ = reduce_and_trig(
            ps_cs[0], CH, cs_sin[:, 0:CH], cs_cos[:, 0:CH], "c", q_on_act=True
        )
        p1a, _ = build_U()
        add_dep_helper(p1a.ins, u0_i.ins, sync=False, reason="order: red0 < U")
        u1_i = None
        for ci in range(1, n_chunks):
            sl = slice(ci * CH, (ci + 1) * CH)
            u1_i, _ = reduce_and_trig(
                ps_cs[ci], CH, cs_sin[:, sl], cs_cos[:, sl], "c", q_on_act=True
            )
        p2a, _ = build_V()
        if u1_i is not None:
            add_dep_helper(p2a.ins, u1_i.ins, sync=False, reason="order: red1 < V")


        # ---- main matmuls + relu + store: per chunk emit all U-matmuls
        #      first, then all V-matmuls (V is built later than U) ----
        obs = {}
        for ci in range(n_chunks):
            sl = slice(ci * CH, (ci + 1) * CH)
            ps_os = []
            for mi, (ms, mr) in enumerate(_m_tiles):
                ps_o = pso.tile((128, CH), F32, tag="pso")
                ps_os.append(ps_o)
                nc.tensor.matmul(
                    ps_o[:mr, :], U[:, ms : ms + mr], cs_cos[:, sl],
                    start=True, stop=False,
                )
            for mi, (ms, mr) in enumerate(_m_tiles):
                ps_o = ps_os[mi]
                nc.tensor.matmul(
                    ps_o[:mr, :], V[:, ms : ms + mr], cs_sin[:, sl],
                    start=False, stop=True,
                )
                if ci == 0:
                    ob_new = sb.tile((128, hw_per), F32, tag=f"ob{mi}")
                    obs[mi] = ob_new
                ob = obs[mi]
                ti = ci * len(_m_tiles) + mi
                if ti % 2 == 0:
                    nc.scalar.activation(
                        ob[:mr, sl], ps_o[:mr, :], AF.Relu, bias=bias_t[0:mr]
                    )
                else:
                    nc.vector.tensor_scalar(
                        ob[:mr, sl], ps_o[:mr, :], bias_t[0:mr], 0.0, A.add, A.max
                    )
                oq = nc.sync if ti % 2 == 0 else nc.scalar
                oq.dma_start(out_d[ms : ms + mr, sl], ob[:mr, sl])

    nc.finalize()
    return nc


_NC = None


def _get_nc():
    global _NC
    if _NC is None:
        _NC = _build_program()
    return _NC


def _make_in_maps(predict_depth, depth_map, W, b):
    pd = np.asarray(predict_depth, np.float32).reshape(N_TOT)
    dm = np.asarray(depth_map, np.float32).reshape(128, 15)
    W = np.asarray(W, np.float32)
    b = np.asarray(b, np.float32)
    we = W[0::2, :]
    wo = W[1::2, :]
    wew = np.ascontiguousarray(np.stack([we, wo], axis=1).reshape(K, 2 * HEADS))
    wow = np.ascontiguousarray(np.stack([wo, we], axis=1).reshape(K, 2 * HEADS))
    bias_rep = np.ascontiguousarray(np.tile(b, 16)[:, None])

    in_maps = []
    for c in range(SN * SH):
        ni, hi = c // SH, c % SH
        pd_sl = pd[ni * n_per : ni * n_per + n_per]
        pd_row = np.zeros((1, n_pad), np.float32)
        pd_row[0, : pd_sl.size] = pd_sl
        in_maps.append(
            {
                "pdrow": pd_row,
                "dmtile": np.ascontiguousarray(dm[hi * dm_rows : (hi + 1) * dm_rows]),
                "wew": wew,
                "wow": wow,
                "bias_rep": bias_rep,
            }
        )
    return in_maps


def _run(inputs, trace=False):
    nc = _get_nc()
    in_maps = _make_in_maps(**inputs)
    res = run_bass_kernel_spmd(
        nc, in_maps, core_ids=list(range(SN * SH)), trace=trace
    )
    out = np.empty((HEADS, N_TOT, HW_TOT), np.float32)
    for c in range(SN * SH):
        ni, hi = c // SH, c % SH
        blk = res.results[c]["out"].reshape(n_pad, HEADS, hw_per).transpose(1, 0, 2)
        n0 = ni * n_per
        n_cnt = min(n_per, N_TOT - n0)
        out[:, n0 : n0 + n_cnt, hi * hw_per : (hi + 1) * hw_per] = blk[:, :n_cnt, :]
    return out, res


def kernel(predict_depth, depth_map, W, b):
    out, _ = _run(
        {"predict_depth": predict_depth, "depth_map": depth_map, "W": W, "b": b}
    )
    return out
